# revision 88
# baseline (speedup 1.0000x reference)
# Cross-attention kernel for Trainium2, 8 NeuronCores.
#
# Reference computation (per batch b):
#   Q = q @ Wq.T + bq ; K = k @ Wk.T + bk ; V = v @ Wv.T + bv      [N, D]
#   per head h (D=1024, H=16, hd=64):
#     S = Qh @ Kh.T * D**-0.5 ; P = softmax(S, axis=-1) ; O = P @ Vh
#   out = concat_h(O) @ Wo.T + bo
#
# Sharding: 8 cores = 4 batches x 2 head-groups (8 heads / 512 channels each).
# Each core computes its batch's projections restricted to its 512 channels,
# attention for its 8 heads, and a partial output projection; the host sums
# the two partials per batch and adds bo.
#
# Precision split:
#   fp8(e4m3) + DoubleRow : q/k inputs, Wq/Wk, Q'/K', the QK matmul.  All
#     errors on this path are damped by scale=1/32 and exp (|S|*scale < ~1),
#     so ~5% relative error on S maps to <1% on the output.
#   bf16                  : v, Wv, Wo, V', exp(S), PV, out-projection.  These
#     feed the output linearly, fp8 would cost ~3-5% there.
#
# DoubleRow layouts (contraction = partitions x 2 interleaved planes):
#   Q'/K' tiles [64p, 2, N] fp8, one per pair, rotating through 2 pool
#     slots.  Head A on partitions 0..32, head B on 32..64; channel c of a
#     head = 32*i + p (i = the DoubleRow plane).  QK per head:
#     lhsT = K'[32p, 2, 128keys], rhs = Q'[32p, 2, 512q] ->
#     S^T[128, 512] in 256 PE cycles.  (DoubleRow matmuls fault on real
#     hardware at partition offsets 64/96, so everything stays at 0/32.)
#   Projections: weights host-permuted to [pair, p, kc, plane, 64]; chains
#     of 4 DoubleRow matmuls (contraction 256 each) -> psum[0:64, 512],
#     bias-added by DVE (or Act on the first-exp critical path) into the
#     Q'/K' tile.
#
# Everything else (softmax-without-max-subtraction, V' with ones column for
# the free rowsum, S^T orientation so exp(S^T) is the PV stationary operand,
# O[n, hd] -> PE-transpose -> O^T, pending-work FIFO pacing the PE stream so
# the Act exp stream never starves) follows the bf16 predecessor of this
# kernel.  The exp stream on Act (~266us) is the binding roofline; PE has
# ~70us of slack which the FIFO spends on V'/PV/out-proj behind the QK feed.

import numpy as np
import ml_dtypes
from collections import deque
from contextlib import ExitStack

import concourse.bacc as bacc
import concourse.bass as bass
import concourse.mybir as mybir
import concourse.tile as tile
from concourse.bass_utils import run_bass_kernel_spmd

F32 = mybir.dt.float32
BF16 = mybir.dt.bfloat16
FP8 = mybir.dt.float8e4
DR = mybir.MatmulPerfMode.DoubleRow
AluOp = mybir.AluOpType
Act = mybir.ActivationFunctionType
E4 = ml_dtypes.float8_e4m3

# full-problem constants
B, N_FULL, M_FULL, D_FULL = 4, 2048, 2048, 1024
HEADS, HD = 16, 64
N_CORES = 8
GROUPS = N_CORES // B  # head groups per batch (2)
E_RING = 38            # exp ring tiles (SBUF budget bound)


def build_program(N, M, D, DH, HD, nbs=512, trn_type="TRN2"):
    """Build the per-core Bass program.

    N: query rows, M: key rows, D: model/contraction dim,
    DH: per-core channels (this group's heads * HD), HD: head dim,
    nbs: query-block size (free dim of S^T tiles).
    """
    P = 128
    H = DH // HD          # local heads
    HP = H // 2           # head pairs == channel chunks
    KC = D // P           # contraction chunks
    CC = DH // P          # channel chunks (== HP)
    MC = M // P           # key chunks
    NB = N // nbs         # query blocks
    NT = nbs // P         # 128-wide n-subtiles per query block
    EB = max(D // 512, 1) # output-proj column blocks
    EBS = min(D, 512)
    QTR = N // 4          # q/k quarter width
    scale = float(D) ** -0.5
    assert CC == HP and H % 2 == 0 and M % P == 0 and N % nbs == 0

    nc = bacc.Bacc(trn_type, target_bir_lowering=False, debug=False,
                   enable_asserts=False, num_devices=1)

    q8d = nc.dram_tensor("q8", [D, N], FP8, kind="ExternalInput")
    k8d = nc.dram_tensor("k8", [D, M], FP8, kind="ExternalInput")
    vT = nc.dram_tensor("vT", [D, M], BF16, kind="ExternalInput")
    # wq8/wk8 are host-laid-out as [pair, partition, kc*128] so a single
    # pair's weights are one dense 1KB-per-partition DMA (no sub-512B
    # descriptor penalty).
    wq8 = nc.dram_tensor("wq8", [DH // P, P, KC * P], FP8,
                         kind="ExternalInput")
    wk8 = nc.dram_tensor("wk8", [DH // P, P, KC * P], FP8,
                         kind="ExternalInput")
    wvT = nc.dram_tensor("wvT", [D, DH], BF16, kind="ExternalInput")
    woT = nc.dram_tensor("woT", [DH, D], BF16, kind="ExternalInput")
    bqk = nc.dram_tensor("bqk", [P, HP * 4], F32, kind="ExternalInput")
    bvb = nc.dram_tensor("bvb", [P, DH], BF16, kind="ExternalInput")
    ident = nc.dram_tensor("ident", [P, P], BF16, kind="ExternalInput")
    out = nc.dram_tensor("out", [N, D], BF16, kind="ExternalOutput")

    with tile.TileContext(nc) as tc, ExitStack() as ctx:
        const = ctx.enter_context(tc.tile_pool(name="const", bufs=1))
        wpool = ctx.enter_context(tc.tile_pool(name="wpool", bufs=1))
        persist = ctx.enter_context(tc.tile_pool(name="persist", bufs=1))
        small = ctx.enter_context(tc.tile_pool(name="small", bufs=4))
        osb_pool = ctx.enter_context(tc.tile_pool(name="osb_pool", bufs=4))
        ob_pool = ctx.enter_context(tc.tile_pool(name="ob_pool", bufs=3))
        qk_pool = ctx.enter_context(tc.tile_pool(name="qk_pool", bufs=1))
        v_pool = ctx.enter_context(tc.tile_pool(name="v_pool", bufs=4))
        qtkt = ctx.enter_context(tc.tile_pool(name="qtkt", bufs=2))
        e_pool = ctx.enter_context(tc.tile_pool(name="e_pool", bufs=E_RING))
        # PSUM: tag "s" = 2 x [P, 2*nbs] (2 banks each), exclusively for S^T
        # tiles; tag "o" = 4 x 1 bank for everything else.
        psum = ctx.enter_context(tc.tile_pool(name="psum", bufs=2,
                                              space="PSUM"))

        # ---- pending-work FIFO: (pe_cost_us, emit_fn) ----
        pending = deque()
        bucket = [0.0]

        def drain(rate_us):
            bucket[0] = min(bucket[0] + rate_us, max(rate_us + 0.1, 0.95))
            while pending and pending[0][0] <= bucket[0]:
                cost, fn = pending.popleft()
                fn()
                bucket[0] -= cost

        def flush():
            while pending:
                pending.popleft()[1]()

        # prefetch the exp table-set on Act immediately (result unused)
        dum = const.tile([P, 8], BF16, name="dum")
        dum2 = const.tile([P, 8], BF16, name="dum2")
        nc.vector.memset(dum, 0.0)
        nc.scalar.activation(dum2, dum, Act.Exp)
        # PE p-state warm-up on garbage data during the DMA window.
        warm = const.tile([P, nbs], BF16, name="warm")
        nc.vector.memset(warm, 0.0)
        for i in range(7):
            wps = psum.tile([P, nbs], F32, name=f"warm{i}", tag="o", bufs=4)
            nc.tensor.matmul(wps, lhsT=warm[:, 0:P], rhs=warm,
                             start=True, stop=True)

        # ---- input DMAs, in first-use order ----
        # critical path to the first exp: q quarter 0, pair-0 wq, biases,
        # k quarter 0, pair-0 wk -- interleaved long/short so the HWDGE
        # issue slots and the serial transfer queue overlap.
        q_r = q8d.ap().rearrange("(kc p) (qt n) -> qt p kc n", p=P, qt=4)
        k_r = k8d.ap().rearrange("(kc p) (qt n) -> qt p kc n", p=P, qt=4)
        q8 = [qk_pool.tile([P, KC, QTR], FP8, name=f"q8_{i}", tag=f"q{i}")
              for i in range(4)]
        k8 = [qk_pool.tile([P, KC, QTR], FP8, name=f"k8_{i}", tag=f"k{i}")
              for i in range(4)]
        wq_sb = wpool.tile([P, HP, KC, 2, 64], FP8)
        wq_r = wq8.ap().rearrange("hp p x -> p hp x").rearrange(
            "p hp (kc i c) -> p hp kc i c", i=2, c=64)
        wk_sb = wpool.tile([P, HP, KC, 2, 64], FP8)
        wk_r = wk8.ap().rearrange("hp p x -> p hp x").rearrange(
            "p hp (kc i c) -> p hp kc i c", i=2, c=64)

        nc.sync.dma_start(q8[0], q_r[0])
        nc.sync.dma_start(wq_sb[:, 0], wq_r[:, 0])
        bqk_sb = const.tile([P, HP * 4], F32)
        nc.sync.dma_start(bqk_sb, bqk.ap())
        bq_v = bqk_sb[:, 0:HP * 2]
        bk_v = bqk_sb[:, HP * 2:HP * 4]
        nc.sync.dma_start(k8[0], k_r[0])
        nc.sync.dma_start(wk_sb[:, 0], wk_r[:, 0])
        nc.sync.dma_start(k8[1], k_r[1])

        # v eighth tiles [P, KC, M/8] bf16; the V' projection runs in two
        # column-half passes (heads 0..H/2-1, then H/2..H-1), each pass
        # re-reading v from DRAM through a 4-slot ring, so the first pass
        # (the one PV(0,0) waits on) is only ~14us of PE work.
        v_r = vT.ap().rearrange("(kc p) (e m) -> e p kc m", p=P, e=8)
        wv_sb = wpool.tile([P, KC * DH], BF16, name="wv_sb", tag="w2")
        wv_v = wv_sb.rearrange("p (kc c) -> p kc c", c=DH)
        nc.sync.dma_start(wv_sb.rearrange("p (kc c) -> p kc c", c=DH),
                          wvT.ap().rearrange("(kc p) c -> p kc c", p=P))
        bv_bc = const.tile([P, DH], BF16)
        nc.sync.dma_start(bv_bc, bvb.ap())
        va = [None] * 8
        vb = [None] * 8

        def v_load(lst, e):
            lst[e] = v_pool.tile([P, KC, M // 8], BF16, name=f"v{e}",
                                 tag="v")
            nc.sync.dma_start(lst[e], v_r[e])

        # interleave: k quarters feed pair-0's inline K-projections (needed
        # at QK steps 4/8/12 of block (0,0)), v eighths feed V-proj pass a.
        v_load(va, 0)
        nc.sync.dma_start(k8[2], k_r[2])
        v_load(va, 1)
        nc.sync.dma_start(k8[3], k_r[3])
        v_load(va, 2)
        nc.sync.dma_start(q8[1], q_r[1])
        nc.sync.dma_start(wq_sb[:, 1:HP], wq_r[:, 1:HP])
        nc.sync.dma_start(wk_sb[:, 1:HP], wk_r[:, 1:HP])
        ident_sb = const.tile([P, P], BF16)
        nc.sync.dma_start(ident_sb, ident.ap())

        def late_dmas():
            # v3+ reuse ring slots (each waits on V-proj pass-a progress),
            # so they are emitted after block (0,0); keep q quarters ahead
            # of the slow ring waits.
            v_load(va, 3)
            nc.sync.dma_start(q8[2], q_r[2])
            v_load(va, 4)
            v_load(va, 5)
            nc.sync.dma_start(q8[3], q_r[3])
            v_load(va, 6)
            v_load(va, 7)

        def late_dmas_b():
            for e in range(8):
                v_load(vb, e)

        # Q'/K' fp8 tiles: one [64, 2, len] tile per pair, rotating through
        # 2 slots (pair hp+2's DMA-free staging waits on pair hp's reads).
        # DoubleRow matmuls only run at partition offsets 0/32 -- offsets
        # 64/96 fault on hardware.
        _pair_tiles = {}

        def pair_loc(hp):
            if hp not in _pair_tiles:
                qt_n = qtkt.tile([64, 2, N], FP8, name=f"qt{hp}", tag="qt")
                kt_n = qtkt.tile([64, 2, M], FP8, name=f"kt{hp}", tag="kt")
                _pair_tiles[hp] = (qt_n, kt_n)
            return _pair_tiles[hp]

        # V' with a ones column appended per head: [m, H*(HD+1)]
        vpp = persist.tile([P, MC, H * (HD + 1)], BF16)
        ont = persist.tile([P, CC, N], BF16)     # normalized O^T
        vpp_v = vpp.rearrange("p mc (h c) -> p mc h c", c=HD + 1)
        wo_holder = [None]

        # ---- work-unit factories ----
        # NOTE: every PSUM accumulation chain must be emitted contiguously
        # in the PE stream (one chain per unit).
        def proj_plane(w_sb, b_v, src, dst, hp, i, b, on_act=False):
            """One DoubleRow projection chain: 64 channels (plane i) of
            pair hp for query/key block b -> psum[0:64] -> dst tile.
            on_act: run the bias-add copy on the (idle) Act engine --
            used on the critical path to the first exp."""
            def f():
                po = psum.tile([P, nbs], F32, name="pj", tag="o", bufs=4)
                for j in range(KC // 2):
                    nc.tensor.matmul(
                        po[0:64, :],
                        lhsT=w_sb[:, hp, 2 * j:2 * j + 2, i, :],
                        rhs=src[b][:, 2 * j:2 * j + 2, :],
                        start=(j == 0), stop=(j == KC // 2 - 1),
                        perf_mode=DR)
                bias = b_v[0:64, 2 * hp + i:2 * hp + i + 1]
                dsl = dst[0:64, i, b * nbs:(b + 1) * nbs]
                if on_act:
                    nc.scalar.add(dsl, po[0:64, :], bias)
                else:
                    nc.vector.tensor_scalar(
                        out=dsl, in0=po[0:64, :], scalar1=bias,
                        scalar2=None, op0=AluOp.add)
            return f

        def proj_units(hp, b):
            """Q then K projection for (pair hp, block b): 4 plane chains."""
            qt_n, kt_n = pair_loc(hp)
            units = []
            for i in range(2):
                units.append((0.55, proj_plane(wq_sb, bq_v, q8, qt_n,
                                               hp, i, b)))
            for i in range(2):
                units.append((0.55, proj_plane(wk_sb, bk_v, k8, kt_n,
                                               hp, i, b)))
            return units

        def vproj_units(vlst, half):
            """V' projection pass for output-column half `half` (heads
            half*H/2 .. (half+1)*H/2-1), reading v eighths from vlst."""
            DHH = DH // 2
            h0 = half * (H // 2)
            cs0 = half * DHH
            units = []
            for mb in range(MC):
                def part(mb=mb):
                    def f():
                        ps = psum.tile([P, DHH], F32, name=f"vp{mb}",
                                       tag="o", bufs=4)
                        vch = vlst[mb // 2]
                        lo = (mb % 2) * P
                        for kc in range(KC):
                            nc.tensor.matmul(
                                ps,
                                lhsT=vch[:, kc, lo:lo + P],
                                rhs=wv_v[:, kc, cs0:cs0 + DHH],
                                start=(kc == 0), stop=(kc == KC - 1))
                        nc.vector.tensor_tensor(
                            out=vpp_v[:, mb, h0:h0 + H // 2, 0:HD],
                            in0=ps.rearrange("p (h c) -> p h c", c=HD),
                            in1=bv_bc[:, cs0:cs0 + DHH].rearrange(
                                "p (h c) -> p h c", c=HD),
                            op=AluOp.add)
                    return f
                units.append((0.9, part()))
            if half == 0:
                def ones():
                    nc.vector.memset(vpp_v[:, :, :, HD:HD + 1], 1.0)
                units.append((0.1, ones))
            else:
                def wo_load():
                    wo_sb = wpool.tile([P, CC * D], BF16, name="wo_sb",
                                       tag="w2")
                    nc.sync.dma_start(
                        wo_sb.rearrange("p (cc e) -> p cc e", e=D),
                        woT.ap().rearrange("(cc p) e -> p cc e", p=P))
                    wo_holder[0] = wo_sb.rearrange("p (cc e) -> p cc e", e=D)
                units.append((0.1, wo_load))
            return units

        def op_unit(ncs, eb):
            def u():
                po = psum.tile([P, EBS], F32, name="po", tag="o", bufs=4)
                for cc in range(CC):
                    nc.tensor.matmul(
                        po, lhsT=ont[:, cc, ncs * P:(ncs + 1) * P],
                        rhs=wo_holder[0][:, cc, eb * EBS:(eb + 1) * EBS],
                        start=(cc == 0), stop=(cc == CC - 1))
                ob = ob_pool.tile([P, EBS], BF16, name="ob",
                                  tag="ob0" if eb == 0 else "ob")
                nc.vector.tensor_copy(ob, po)
                nc.sync.dma_start(
                    out.ap()[ncs * P:(ncs + 1) * P,
                             eb * EBS:(eb + 1) * EBS], ob)
            return (0.9, u)

        # last-block out-proj: head pairs 0..CC-2 are accumulated early
        # into bf16 partials; after the final PV only one matmul (pair
        # CC-1) + an add remain per column block.
        part_t = {}

        def op_pre_unit(ncs, eb):
            def u():
                po = psum.tile([P, EBS], F32, name="pop", tag="o", bufs=4)
                for cc in range(CC - 1):
                    nc.tensor.matmul(
                        po, lhsT=ont[:, cc, ncs * P:(ncs + 1) * P],
                        rhs=wo_holder[0][:, cc, eb * EBS:(eb + 1) * EBS],
                        start=(cc == 0), stop=(cc == CC - 2))
                pt = persist.tile([P, EBS], BF16, name=f"part{ncs}_{eb}",
                                  tag=f"part{ncs}_{eb}")
                nc.vector.tensor_copy(pt, po)
                part_t[(ncs, eb)] = pt
            return (0.75, u)

        tail_k = [0]

        def op_tail_unit(ncs, eb):
            # alternate the +partial and staging between the DVE path
            # (tensor_tensor add) and the Act path (identity-matmul
            # accumulate on PE, then scalar copy), and the out-DMA issue
            # between the SP and Act HWDGE queues, so no single engine
        # serializes the post-exp tail.
            def u():
                k = tail_k[0]
                tail_k[0] += 1
                po = psum.tile([P, EBS], F32, name="pot", tag="o", bufs=4)
                cc = CC - 1
                ob = ob_pool.tile([P, EBS], BF16, name="ob",
                                  tag="ob0" if eb == 0 else "ob")
                if k < 5:
                    # first half: DVE adds the staged partial
                    nc.tensor.matmul(
                        po, lhsT=ont[:, cc, ncs * P:(ncs + 1) * P],
                        rhs=wo_holder[0][:, cc, eb * EBS:(eb + 1) * EBS],
                        start=True, stop=True)
                    nc.vector.tensor_tensor(
                        out=ob, in0=po, in1=part_t[(ncs, eb)],
                        op=AluOp.add)
                else:
                    # second half: +partial via identity-matmul on the PE,
                    # staging copy on the (post-exp idle) Act engine, so
                    # the tail's last copies don't queue behind DVE
                    nc.tensor.matmul(
                        po, lhsT=ont[:, cc, ncs * P:(ncs + 1) * P],
                        rhs=wo_holder[0][:, cc, eb * EBS:(eb + 1) * EBS],
                        start=True, stop=False)
                    nc.tensor.matmul(
                        po, lhsT=ident_sb, rhs=part_t[(ncs, eb)],
                        start=False, stop=True)
                    nc.scalar.copy(ob, po)
                nc.sync.dma_start(
                    out.ap()[ncs * P:(ncs + 1) * P,
                             eb * EBS:(eb + 1) * EBS], ob)
            return (0.35, u)

        def pv_units(hp, b, e_tiles):
            """8 PV chains (O[n, hd+1] orientation) + per-chain normalize,
            then 4 two-head 128x128 transposes into ont.  For the last head
            pair each transpose is followed by that n-tile's output
            projection, so the tail drains column by column."""
            hA, hB = 2 * hp, 2 * hp + 1
            last = hp == HP - 1 and b == NB - 1
            units = []
            post = []  # per-j transpose/out-proj units
            tps = []   # last block: transposes, hoisted ahead of tails
            boxes = [dict() for _ in range(NT)]
            for j in range(NT):
                for h_i, h in ((0, hA), (1, hB)):
                    def chain(j=j, h_i=h_i, h=h, box=boxes[j]):
                        if "osb" not in box:
                            box["osb"] = osb_pool.tile([P, P], BF16,
                                                       name="osb", tag="osb")
                        oc = psum.tile([P, HD + 1], F32, name="oc", tag="o",
                                       bufs=4)
                        for mc in range(MC):
                            nc.tensor.matmul(
                                oc,
                                lhsT=e_tiles[mc][
                                    :, h_i * nbs + j * P:
                                    h_i * nbs + (j + 1) * P],
                                rhs=vpp_v[:, mc, h, :],
                                start=(mc == 0), stop=(mc == MC - 1))
                        rs = small.tile([P, 1], F32, name="rs", tag="rs")
                        nc.vector.reciprocal(rs, oc[:, HD:HD + 1])
                        if last:
                            nc.scalar.activation(
                                box["osb"][:, h_i * HD:(h_i + 1) * HD],
                                oc[:, 0:HD], Act.Copy, scale=rs)
                        else:
                            nc.vector.tensor_scalar(
                                out=box["osb"][:, h_i * HD:(h_i + 1) * HD],
                                in0=oc[:, 0:HD], scalar1=rs, scalar2=None,
                                op0=AluOp.mult)
                    units.append((0.5, chain))
                def transp(j=j, box=boxes[j]):
                    tp = psum.tile([P, P], BF16, name="tp", tag="o", bufs=4)
                    nc.tensor.transpose(tp, box["osb"], ident_sb)
                    nc.vector.tensor_copy(
                        ont[:, hp, b * nbs + j * P:b * nbs + (j + 1) * P], tp)
                if last:
                    # all transposes ahead of all tail units so the DVE
                    # ont-copies aren't queued behind the tail adds
                    tps.append((0.1, transp))
                else:
                    post.append((0.1, transp))
                if hp == HP - 1:
                    for eb in range(EB):
                        if last:
                            post.append(op_tail_unit(b * NT + j, eb))
                        else:
                            post.append(op_unit(b * NT + j, eb))
            units.extend(tps)
            units.extend(post)
            return units

        # ---- main loop ----
        for hp in range(HP):
            qt_hp, kt_hp = pair_loc(hp)
            for b in range(NB):
                first = hp == 0 and b == 0
                if hp + 1 < HP and not first:
                    # stage the next head-pair's projection (pair 1's four
                    # blocks ride hp0's blocks 1-3: two in b=1).
                    stage_is = ({1: [0, 1], 2: [2], 3: [3]}.get(b, [])
                                if hp == 0 else [b])
                    for i in stage_is:
                        pending.extend(proj_units(hp + 1, i))
                nsl = slice(b * nbs, (b + 1) * nbs)
                e_tiles = []
                for mc in range(MC):
                    if first:
                        # pair 0 projects itself: Q+K block 0 up front, then
                        # K blocks 1-3 as their DMAs land; deferred Q blocks
                        # 1-3 are emitted at the end of blocks 0-2.
                        if mc == 0:
                            for i2 in range(2):
                                proj_plane(wq_sb, bq_v, q8, qt_hp,
                                           0, i2, 0)()
                            for i2 in range(2):
                                # K copies on Act: parallel to Q's DVE
                                # copies on the first-exp critical path
                                proj_plane(wk_sb, bk_v, k8, kt_hp,
                                           0, i2, 0, on_act=True)()
                        elif (mc % (MC // NB) == 2
                              and mc // (MC // NB) + 1 < NB):
                            # K planes only (Q deferred); emitted two QK
                            # steps before their block is needed
                            i = mc // (MC // NB) + 1
                            for i2 in range(2):
                                proj_plane(wk_sb, bk_v, k8, kt_hp,
                                           0, i2, i)()
                    else:
                        rate = {(0, 1): 1.1, (0, 2): 1.0, (0, 3): 0.95,
                                (1, 0): 0.9, (1, 1): 0.9}.get((hp, b))
                        if rate is None:
                            rate = 0.8 if hp < HP - 1 else 1.0
                        drain(rate)
                    s = psum.tile([P, 2 * nbs], F32, name="s", tag="s",
                                  bufs=2)
                    nc.tensor.matmul(
                        s[:, 0:nbs],
                        lhsT=kt_hp[0:32, :, mc * P:(mc + 1) * P],
                        rhs=qt_hp[0:32, :, nsl],
                        start=True, stop=True, perf_mode=DR)
                    nc.tensor.matmul(
                        s[:, nbs:2 * nbs],
                        lhsT=kt_hp[32:64, :, mc * P:(mc + 1) * P],
                        rhs=qt_hp[32:64, :, nsl],
                        start=True, stop=True, perf_mode=DR)
                    e = e_pool.tile([P, 2 * nbs], BF16, name="e", tag="e")
                    nc.scalar.activation(e, s, Act.Exp, scale=scale)
                    e_tiles.append(e)
                if first:
                    late_dmas()
                    pending.extend(vproj_units(va, 0))
                if hp == 0 and b < NB - 1:
                    # deferred Q-proj for pair 0, block b+1 (its q quarter
                    # has landed by the end of block b)
                    for i2 in range(2):
                        proj_plane(wq_sb, bq_v, q8, qt_hp, 0, i2, b + 1)()
                pending.extend(pv_units(hp, b, e_tiles))
                if hp == HP - 2 and b == NB - 1:
                    # stage the last block's out-proj partials (pairs
                    # 0..CC-2) right after PV(HP-2, NB-1) -- they drain
                    # early in the last head-pair's stream
                    for j in range(NT):
                        for eb in range(EB):
                            pending.append(op_pre_unit((NB - 1) * NT + j,
                                                       eb))
                if hp == 0 and b == 2:
                    # V' pass b (heads H/2..H-1) rides hp0's tail + hp1;
                    # its first consumer is PV(2,0), far out.
                    late_dmas_b()
                    pending.extend(vproj_units(vb, 1))
        flush()  # emit the tail (last block's PV + out-proj)

    nc.compile()
    return nc


_PROGRAM = None


def _get_program():
    global _PROGRAM
    if _PROGRAM is None:
        _PROGRAM = build_program(N_FULL, M_FULL, D_FULL,
                                 D_FULL // GROUPS, HD)
    return _PROGRAM


def _perm(DH):
    """Channel permutation: perm[hp, i, j] = channel index within the
    group's DH channels."""
    HP = DH // 128
    perm = np.zeros((HP, 2, 64), np.int64)
    for hp in range(HP):
        for i in range(2):
            for j in range(64):
                if j < 32:
                    c = 32 * i + j          # head A
                else:
                    c = 64 + 32 * i + (j - 32)  # head B
                perm[hp, i, j] = 128 * hp + c
    return perm


def _prep_inputs(q, k, v, Wq, bq, Wk, bk, Wv, bv, Wo, bo):
    """Host-side shard + layout prep -> per-core input dicts."""
    bf = ml_dtypes.bfloat16
    DH = D_FULL // GROUPS
    f32 = np.float32

    q8T = [np.ascontiguousarray(np.asarray(q[b], f32).T).astype(E4)
           for b in range(B)]
    k8T = [np.ascontiguousarray(np.asarray(k[b], f32).T).astype(E4)
           for b in range(B)]
    vTb = [np.ascontiguousarray(np.asarray(v[b], f32).T).astype(bf)
           for b in range(B)]
    WqT = np.asarray(Wq, f32).T
    WkT = np.asarray(Wk, f32).T
    WvT = np.asarray(Wv, f32).T
    WoT = np.asarray(Wo, f32).T
    bq = np.asarray(bq, f32); bk = np.asarray(bk, f32)
    bv = np.asarray(bv, f32)
    ident = np.eye(128, dtype=bf)
    perm = _perm(DH)  # [HP, 2, 64] channel-in-group indices
    perm_flat = perm.reshape(-1)

    per_g = []
    for g in range(GROUPS):
        cs = slice(g * DH, (g + 1) * DH)
        KC, HP = D_FULL // 128, DH // 128
        wq_g = WqT[:, cs][:, perm_flat]          # [D, DH] permuted
        wk_g = WkT[:, cs][:, perm_flat]
        # device layout [pair, partition, kc*128]
        wq_g = np.ascontiguousarray(
            wq_g.reshape(KC, 128, HP, 128).transpose(2, 1, 0, 3)
            .reshape(HP, 128, KC * 128))
        wk_g = np.ascontiguousarray(
            wk_g.reshape(KC, 128, HP, 128).transpose(2, 1, 0, 3)
            .reshape(HP, 128, KC * 128))
        bq_g = bq[cs][perm[:, :, :]].transpose(2, 0, 1)  # [64, HP, 2]
        bk_g = bk[cs][perm[:, :, :]].transpose(2, 0, 1)
        bq_dev = np.concatenate([bq_g, bq_g], axis=0).reshape(128, -1)
        bk_dev = np.concatenate([bk_g, bk_g], axis=0).reshape(128, -1)
        bvb = np.broadcast_to(bv[cs].reshape(1, DH), (128, DH))
        per_g.append({
            "wq8": wq_g.astype(E4),
            "wk8": wk_g.astype(E4),
            "wvT": np.ascontiguousarray(WvT[:, cs]).astype(bf),
            "woT": np.ascontiguousarray(WoT[cs, :]).astype(bf),
            "bqk": np.ascontiguousarray(
                np.concatenate([bq_dev, bk_dev], axis=1)),
            "bvb": np.ascontiguousarray(bvb).astype(bf),
            "ident": ident,
        })

    in_maps = []
    for b in range(B):
        for g in range(GROUPS):
            m = {"q8": q8T[b], "k8": k8T[b], "vT": vTb[b]}
            m.update(per_g[g])
            in_maps.append(m)
    return in_maps


LAST_RESULT = None


def kernel(q, k, v, Wq, bq, Wk, bk, Wv, bv, Wo, bo):
    global LAST_RESULT
    nc = _get_program()
    in_maps = _prep_inputs(q, k, v, Wq, bq, Wk, bk, Wv, bv, Wo, bo)
    res = run_bass_kernel_spmd(nc, in_maps, core_ids=list(range(N_CORES)))
    LAST_RESULT = res
    bo = np.asarray(bo, np.float32)
    outs = [res.results[b * GROUPS]["out"].astype(np.float32)
            + res.results[b * GROUPS + 1]["out"].astype(np.float32)
            + bo for b in range(B)]
    return np.stack(outs).astype(np.float32)
